# revision 15
# baseline (speedup 1.0000x reference)
# Causal self-attention (B=4, T=2048, C=1024, 16 heads) on 8 NeuronCores.
# Sharding: core = (batch b = core//2) x (head-group hg = core%2, 8 heads each).
# Each core computes its 8 heads' attention for its batch plus the row-slice of
# the output projection; the host sums the two partial projections per batch.
#
# Self-contained: hardcodes shapes; builds + compiles the Bass program once.

import contextlib

import numpy as np
import ml_dtypes

import concourse.bass as bass
import concourse.tile as tile
from concourse import bacc, mybir
from concourse.bass import AP
from concourse.bass_utils import run_bass_kernel_spmd

F32 = mybir.dt.float32
BF16 = mybir.dt.bfloat16
EXP = mybir.ActivationFunctionType.Exp

B, T, C = 4, 2048, 1024
NH, HS = 16, 64
NHPC = 8          # heads per core
D = NHPC * HS     # 512: per-core qkv width
NCORES = 8
TT = T // 128     # 16 token tiles
TC = T // 512     # 4 token chunks
CT = C // 128     # 8 contraction tiles
DT = D // 128     # 4 d-tiles of qT/kT (= head pairs)
PW = 256          # head-pair stride in v tile: [1|0*63|v_e(64)] [1|0*63|v_o(64)]

_cache = {}


def _build():
    nc = bacc.Bacc("TRN2", target_bir_lowering=False, debug=False,
                   num_devices=NCORES)

    xT = nc.dram_tensor("xT", [C, T], BF16, kind="ExternalInput")
    wq = nc.dram_tensor("wq", [C, D], BF16, kind="ExternalInput")
    wk = nc.dram_tensor("wk", [C, D], BF16, kind="ExternalInput")
    wv = nc.dram_tensor("wv", [C, D], BF16, kind="ExternalInput")
    wp = nc.dram_tensor("wp", [D, C], BF16, kind="ExternalInput")
    bq = nc.dram_tensor("bq", [128, DT], F32, kind="ExternalInput")
    bk = nc.dram_tensor("bk", [128, DT], F32, kind="ExternalInput")
    bvbc = nc.dram_tensor("bvbc", [128, D], F32, kind="ExternalInput")
    ypT = nc.dram_tensor("ypT", [C, T], BF16, kind="ExternalOutput")

    with tile.TileContext(nc) as tc, contextlib.ExitStack() as ctx:
        cpool = ctx.enter_context(tc.tile_pool(name="consts", bufs=1))
        xpool = ctx.enter_context(tc.tile_pool(name="x", bufs=1))
        wpool = ctx.enter_context(tc.tile_pool(name="w", bufs=1))
        qkpool = ctx.enter_context(tc.tile_pool(name="qk", bufs=1))
        vpool = ctx.enter_context(tc.tile_pool(name="v", bufs=1))
        opool = ctx.enter_context(tc.tile_pool(name="oT", bufs=1))
        ptpool = ctx.enter_context(tc.tile_pool(name="pt", bufs=8))
        ypool = ctx.enter_context(tc.tile_pool(name="y", bufs=4))
        rrpool = ctx.enter_context(tc.tile_pool(name="rr", bufs=2))
        rvpool = ctx.enter_context(tc.tile_pool(name="rv", bufs=2))
        pfpool = ctx.enter_context(tc.tile_pool(name="pf", bufs=1))
        qkv_ps = ctx.enter_context(
            tc.tile_pool(name="qkv_ps", bufs=2, space="PSUM"))
        s_ps = ctx.enter_context(
            tc.tile_pool(name="s_ps", bufs=2, space="PSUM"))
        o_ps = ctx.enter_context(
            tc.tile_pool(name="o_ps", bufs=2, space="PSUM"))

        # ---- input DMAs: wv0 first (gates the warm-up matmuls), then the
        # tiny biases, then by first compute use ----
        wvt = [wpool.tile([128, D], BF16, tag=f"wv{i}", name=f"wv{i}")
               for i in range(CT)]
        nc.sync.dma_start(wvt[0][:], wv.ap()[0:128, :])
        bqt = cpool.tile([128, DT], F32, tag="bq")
        bkt = cpool.tile([128, DT], F32, tag="bk")
        bvt = cpool.tile([128, D], F32, tag="bv")
        nc.sync.dma_start(bqt[:], bq.ap())
        nc.scalar.dma_start(bkt[:], bk.ap())
        nc.sync.dma_start(bvt[:], bvbc.ap())
        for i in range(1, CT):
            (nc.sync if i % 2 == 0 else nc.scalar).dma_start(
                wvt[i][:], wv.ap()[i * 128:(i + 1) * 128, :])
        xt = [xpool.tile([128, T], BF16, tag=f"xT{i}", name=f"xT{i}")
              for i in range(CT)]
        wqt = [wpool.tile([128, D], BF16, tag=f"wq{i}", name=f"wq{i}")
               for i in range(CT)]
        wkt = [wpool.tile([128, D], BF16, tag=f"wk{i}", name=f"wk{i}")
               for i in range(CT)]

        def dma_x_chunk(cch):
            for i in range(CT):
                (nc.sync if i % 2 == 0 else nc.scalar).dma_start(
                    xt[i][:, cch * 512:(cch + 1) * 512],
                    xT.ap()[i * 128:(i + 1) * 128, cch * 512:(cch + 1) * 512])
        dma_x_chunk(0)
        for i in range(CT):
            nc.sync.dma_start(wqt[i][:], wq.ap()[i * 128:(i + 1) * 128, :])
            nc.scalar.dma_start(wkt[i][:], wk.ap()[i * 128:(i + 1) * 128, :])
        for cch in range(1, TC):
            dma_x_chunk(cch)
        wpt = [wpool.tile([128, C], BF16, tag=f"wp{i}", name=f"wp{i}")
               for i in range(DT)]
        for i in range(DT):
            (nc.sync if i % 2 == 0 else nc.scalar).dma_start(
                wpt[i][:], wp.ap()[i * 128:(i + 1) * 128, :])

        # ---- one-time consts ----
        # tri2: two side-by-side [128,128] upper-tri (keep query>=key) masks
        ones_t = cpool.tile([128, 256], F32, tag="ones_t")
        nc.vector.memset(ones_t[:], 1.0)
        trif = cpool.tile([128, 256], F32, tag="trif")
        nc.gpsimd.affine_select(
            trif[:].rearrange("p (b q) -> p b q", b=2),
            ones_t[:].rearrange("p (b q) -> p b q", b=2),
            pattern=[[0, 2], [1, 128]],
            compare_op=mybir.AluOpType.is_ge, fill=0.0,
            base=0, channel_multiplier=-1)
        tri2 = cpool.tile([128, 256], BF16, tag="tri2")
        nc.vector.tensor_copy(tri2[:], trif[:])

        # ---- warm-up matmuls: keep PE busy (and HAM warm) during input DMA;
        # operand is a memset tile so warm-up never waits on any input DMA
        wrm = cpool.tile([128, 512], BF16, tag="wrm")
        nc.vector.memset(wrm[:], 0.125)
        warm_ps = qkv_ps.tile([128, 512], F32, tag="qkv", name="warmps")
        for w in range(16):
            nc.tensor.matmul(
                warm_ps[:], wrm[:, 0:128], wrm[:, 0:512],
                start=True, stop=True, skip_group_check=True)

        # ---- v tiles per head pair: even block [1|0*63|v_e(64)],
        # odd block [v_o(64)|1|0*63] so odd-head AV output lands on psum
        # partitions 0:64 (aligned with oT rows 0:64 -> no shift DMA)
        vt = [vpool.tile([128, DT * PW], BF16, tag=f"v{j}", name=f"v{j}")
              for j in range(TT)]
        for j in range(TT):
            vre = vt[j][:].rearrange("p (h e w) -> p h e w", e=2, w=128)
            nc.vector.memset(vre[:, :, 0, 0:64], 0.0)
            nc.vector.memset(vre[:, :, 0, 0:1], 1.0)
            nc.vector.memset(vre[:, :, 1, 64:128], 0.0)
            nc.vector.memset(vre[:, :, 1, 64:65], 1.0)

        def v_units(j):
            # one emission unit per matmul; state shared via closure cell
            cell = {}

            def mk(ct):
                def u():
                    if ct == 0:
                        cell["ps"] = qkv_ps.tile([128, D], F32, tag="qkv",
                                                 name="qkvps")
                    nc.tensor.matmul(
                        cell["ps"][:], xt[ct][:, j * 128:(j + 1) * 128],
                        wvt[ct][:], start=(ct == 0), stop=(ct == CT - 1))
                    if ct == CT - 1:
                        ps = cell["ps"]
                        vre = vt[j][:].rearrange("p (h e w) -> p h e w",
                                                 e=2, w=128)
                        pse = ps[:].rearrange("p (h e x) -> p h e x",
                                              e=2, x=64)
                        bve = bvt[:].rearrange("p (h e x) -> p h e x",
                                               e=2, x=64)
                        nc.vector.tensor_add(vre[:, :, 0, 64:128],
                                             pse[:, :, 0, :], bve[:, :, 0, :])
                        nc.vector.tensor_add(vre[:, :, 1, 0:64],
                                             pse[:, :, 1, :], bve[:, :, 1, :])
                return u
            return [mk(ct) for ct in range(CT)]

        # ---- qT/kT/oT tiles ----
        qT = [qkpool.tile([128, T], BF16, tag=f"q{d}", name=f"q{d}")
              for d in range(DT)]
        kT = [qkpool.tile([128, T], BF16, tag=f"k{d}", name=f"k{d}")
              for d in range(DT)]
        oT = [opool.tile([128, T], BF16, tag=f"oT{d}", name=f"oT{d}")
              for d in range(DT)]

        def qk_units(hp, idx):
            c, which = idx // 2, idx % 2
            wt_, bt_, out = ((wqt, bqt, qT), (wkt, bkt, kT))[which]
            cell = {}

            def mk(ct):
                def u():
                    if ct == 0:
                        cell["ps"] = qkv_ps.tile([128, 512], F32, tag="qkv",
                                                 name="qkvps")
                    nc.tensor.matmul(
                        cell["ps"][:], wt_[ct][:, hp * 128:(hp + 1) * 128],
                        xt[ct][:, c * 512:(c + 1) * 512],
                        start=(ct == 0), stop=(ct == CT - 1))
                    if ct == CT - 1:
                        nc.vector.tensor_scalar_add(
                            out[hp][:, c * 512:(c + 1) * 512], cell["ps"][:],
                            bt_[:, hp:hp + 1])
                return u
            return [mk(ct) for ct in range(CT)]

        def proj_units(c):
            units = []
            for o in range(CT):
                cell = {}

                def mk(o, cell, hp):
                    def u():
                        if hp == 0:
                            cell["ps"] = qkv_ps.tile([128, 512], F32,
                                                     tag="qkv", name="qkvps")
                        nc.tensor.matmul(
                            cell["ps"][:], wpt[hp][:, o * 128:(o + 1) * 128],
                            oT[hp][:, c * 512:(c + 1) * 512],
                            start=(hp == 0), stop=(hp == DT - 1))
                        if hp == DT - 1:
                            ys = ypool.tile([128, 512], BF16, tag="y",
                                            name="ys")
                            nc.vector.tensor_copy(ys[:], cell["ps"][:])
                            nc.sync.dma_start(
                                ypT.ap()[o * 128:(o + 1) * 128,
                                         c * 512:(c + 1) * 512], ys[:])
                    return u
                units += [mk(o, cell, hp) for hp in range(DT)]
            return units

        # last chunk: hp0-2 proj partials prefilled as fillers inside the
        # final attention (psum rotates through qkv_ps; partial -> sbuf f32),
        # leaving only the 8 hp3 matmuls + DVE adds for the true tail
        partial_t = [pfpool.tile([128, 512], F32, tag=f"pf{o}", name=f"pf{o}")
                     for o in range(CT)]

        def proj_partial_units(c):
            units = []
            for o in range(CT):
                cell = {}

                def mk(o, cell, hp):
                    def u():
                        if hp == 0:
                            cell["ps"] = qkv_ps.tile([128, 512], F32,
                                                     tag="qkv", name="ppps")
                        nc.tensor.matmul(
                            cell["ps"][:], wpt[hp][:, o * 128:(o + 1) * 128],
                            oT[hp][:, c * 512:(c + 1) * 512],
                            start=(hp == 0), stop=(hp == DT - 2))
                        if hp == DT - 2:
                            nc.vector.tensor_copy(partial_t[o][:],
                                                  cell["ps"][:])
                    return u
                units += [mk(o, cell, hp) for hp in range(DT - 1)]
            return units

        pending = []

        def flush_pending():
            # deferred normalize: by now the broadcasts are long done, so
            # these DVE ops never stall the queue (no head-of-line blocking)
            while pending:
                hp_, cs_, op0_, op1_, rv_ = pending.pop(0)
                nc.vector.tensor_mul(oT[hp_][64:128, cs_], op0_[64:128, :],
                                     rv_[64:128, 0:512])
                nc.vector.tensor_mul(oT[hp_][0:64, cs_], op1_[0:64, :],
                                     rv_[0:64, 512:1024])

        def emit_scores(c, hp, jt):
            t = jt - 4 * c              # >= 0 on diagonal-block tiles
            off = 128 * t if t > 0 else 0
            sp = s_ps.tile([128, 1024], F32, tag="s", name="sps")
            for half in range(2):
                nc.tensor.matmul(
                    sp[:, half * 512 + off:(half + 1) * 512],
                    kT[hp][half * 64:(half + 1) * 64,
                           jt * 128:(jt + 1) * 128],
                    qT[hp][half * 64:(half + 1) * 64,
                           c * 512 + off:(c + 1) * 512],
                    start=True, stop=True)
            return sp, t, off

        def emit_exp_mask(sp, t, off):
            pt = ptpool.tile([128, 1024], BF16, tag="pt", name="pt")
            if off == 0:
                nc.scalar.activation(pt[:], sp[:], EXP, scale=0.125)
            else:
                nc.scalar.activation(
                    pt[:].rearrange("p (h q) -> p h q", h=2)[:, :, off:],
                    sp[:].rearrange("p (h q) -> p h q", h=2)[:, :, off:],
                    EXP, scale=0.125)
            if t >= 0:
                # triangle mask on the first 128 valid queries of each head
                nc.vector.tensor_mul(
                    pt[:].rearrange("p (h q) -> p h q",
                                    h=2)[:, :, off:off + 128],
                    pt[:].rearrange("p (h q) -> p h q",
                                    h=2)[:, :, off:off + 128],
                    tri2[:].rearrange("p (b q) -> p b q", b=2))
            return pt

        def emit_av(hp, jt, njt, pt, t, off, op0, op1):
            for half, op in ((0, op0), (1, op1)):
                # stationary 128 cols (FWL): [ones|zeros63|v(64)]
                st = vt[jt][:, hp * PW + 128 * half:
                            hp * PW + 128 * half + 128]
                nc.tensor.matmul(
                    op[:, off:512], st,
                    pt[:, half * 512 + off:(half + 1) * 512],
                    start=(jt == 0), stop=(jt == njt - 1),
                    skip_group_check=(t > 0))

        def attention(c, hp, fillers):
            njt = 4 * c + 4
            cs = slice(c * 512, (c + 1) * 512)
            flush_pending()
            op0 = o_ps.tile([128, 512], F32, tag="o", name="ops")
            op1 = o_ps.tile([128, 512], F32, tag="o", name="ops")
            # spread filler matmul units (next qk/v/proj) across the jt loop
            # so the PE never starves while waiting on the scalar exp
            nf = len(fillers)
            done = 0
            for jt in range(njt):
                sp, t, off = emit_scores(c, hp, jt)
                pt = emit_exp_mask(sp, t, off)
                # fillers go BETWEEN scores and AV: the in-order PE queue then
                # has ready matmuls to run while exp(jt) computes
                want = nf * (jt + 1) // njt
                while done < want:
                    fillers[done]()
                    done += 1
                emit_av(hp, jt, njt, pt, t, off, op0, op1)
            while done < nf:
                fillers[done]()
                done += 1

            # normalize: even rowsum at partition 0 of op0, odd rowsum at
            # partition 64 of op1; y halves land lane-aligned with oT.
            # partition_broadcast only reads partition 0, so shuffle the odd
            # recip row from partition group 16 (=64..67) down to group 0
            # recip/broadcast only operate correctly at partition 0, so the
            # odd rowsum (psum partition 64) goes: DVE copy (lane 64) ->
            # SBUF->SBUF DMA row down to partition 0 -> recip -> broadcast
            rr = rrpool.tile([128, 1536], F32, tag="rr", name="rr")
            nc.vector.tensor_copy(rr[64:65, 512:1024], op1[64:65, :])
            nc.gpsimd.dma_start(rr[0:1, 512:1024], rr[64:65, 512:1024])
            nc.vector.reciprocal_approx_fast(rr[0:1, 0:512], op0[0:1, :])
            nc.vector.reciprocal_approx_fast(rr[0:1, 1024:1536],
                                             rr[0:1, 512:1024])
            rv = rvpool.tile([128, 1024], F32, tag="rv", name="rv")
            nc.gpsimd.partition_broadcast(rv[:, 0:512], rr[0:1, 0:512])
            nc.gpsimd.partition_broadcast(rv[0:64, 512:1024],
                                          rr[0:1, 1024:1536])
            pending.append((hp, cs, op0, op1, rv))

        # ---- main loop: chunk-outer; filler units (next qk, next-chunk v,
        # prev-chunk proj) are spread inside each attention's jt loop so the
        # in-order PE queue always has ready matmuls while exp runs
        for j in range(4):
            for u in v_units(j):
                u()
        for u in qk_units(0, 0) + qk_units(0, 1):
            u()
        proj_rest = []
        for c in range(TC):
            for hp in range(DT):
                # qk/v fillers first (no oT dependency); proj fillers last so
                # they never block the in-order PE queue on the oT chain
                fillers = []
                if hp < DT - 1:
                    fillers += qk_units(hp + 1, 2 * c)
                    fillers += qk_units(hp + 1, 2 * c + 1)
                    if c == TC - 1 and hp == 2:
                        fillers += proj_rest
                        proj_rest = []
                else:
                    if c + 1 < TC:
                        for j in range(4 * (c + 1), 4 * (c + 1) + 4):
                            fillers += v_units(j)
                        fillers += qk_units(0, 2 * (c + 1))
                        fillers += qk_units(0, 2 * (c + 1) + 1)
                        fillers += proj_rest
                        proj_rest = []
                    else:
                        # last attention: prefill last-chunk proj hp0-2
                        fillers += proj_partial_units(c)
                if hp == 1 and c > 0:
                    p = proj_units(c - 1)
                    if c == TC - 1:
                        fillers += p[:16]
                        proj_rest = p[16:]
                    else:
                        fillers += p
                attention(c, hp, fillers)
        flush_pending()
        # keep PE busy (and HAM warm) while the last attention's normalize
        # chain runs; these have no dependencies and drain before the hp3
        # matmuls below become ready
        for w in range(12):
            warm2 = qkv_ps.tile([128, 512], F32, tag="qkv", name="warm2")
            nc.tensor.matmul(
                warm2[:], wrm[:, 0:128], wrm[:, 0:512],
                start=True, stop=True, skip_group_check=True)
        # tail: per o-block one hp3 matmul, then DVE add with the prefilled
        # hp0-2 partial straight to bf16 output; DMAs alternate queues
        cl = TC - 1
        for o in range(CT):
            tps = qkv_ps.tile([128, 512], F32, tag="qkv", name="tps")
            nc.tensor.matmul(
                tps[:], wpt[DT - 1][:, o * 128:(o + 1) * 128],
                oT[DT - 1][:, cl * 512:(cl + 1) * 512], start=True, stop=True)
            ys = ypool.tile([128, 512], BF16, tag="y", name="ys")
            nc.vector.tensor_add(ys[:], tps[:], partial_t[o][:])
            (nc.sync if o % 2 == 0 else nc.scalar).dma_start(
                ypT.ap()[o * 128:(o + 1) * 128,
                         cl * 512:(cl + 1) * 512], ys[:])

    nc.compile()
    return nc


def _shard_inputs(x, Wk, bk, Wq, bq, Wv, bv, Wp, bp):
    bf = ml_dtypes.bfloat16
    in_maps = []
    for core in range(NCORES):
        b, hg = core // 2, core % 2
        sl = slice(hg * D, (hg + 1) * D)
        # oT rows per head-pair block: [odd head (64) | even head (64)]
        wp_perm = Wp[sl, :].reshape(DT, 2, 64, C)[:, ::-1].reshape(D, C)
        in_maps.append({
            "xT": np.ascontiguousarray(x[b].T).astype(bf),
            "wq": np.ascontiguousarray(Wq[:, sl]).astype(bf),
            "wk": np.ascontiguousarray(Wk[:, sl]).astype(bf),
            "wv": np.ascontiguousarray(Wv[:, sl]).astype(bf),
            "wp": np.ascontiguousarray(wp_perm).astype(bf),
            "bq": np.ascontiguousarray(
                bq[sl].reshape(DT, 128).T).astype(np.float32),
            "bk": np.ascontiguousarray(
                bk[sl].reshape(DT, 128).T).astype(np.float32),
            "bvbc": np.ascontiguousarray(
                np.broadcast_to(bv[sl], (128, D))).astype(np.float32),
        })
    return in_maps


def kernel(x, Wk, bk, Wq, bq, Wv, bv, Wp, bp, _trace=False, _trace_kwargs=None):
    x, Wk, bk, Wq, bq, Wv, bv, Wp, bp = [
        np.asarray(a) for a in (x, Wk, bk, Wq, bq, Wv, bv, Wp, bp)]
    if "nc" not in _cache:
        _cache["nc"] = _build()
    nc = _cache["nc"]
    in_maps = _shard_inputs(x, Wk, bk, Wq, bq, Wv, bv, Wp, bp)
    kw = dict(_trace_kwargs or {})
    res = run_bass_kernel_spmd(nc, in_maps, core_ids=list(range(NCORES)),
                               trace=_trace, **kw)
    out = np.empty((B, T, C), np.float32)
    for b in range(B):
        yp = (res.results[2 * b]["ypT"].astype(np.float32)
              + res.results[2 * b + 1]["ypT"].astype(np.float32))
        out[b] = yp.T + bp[None, :]
    if _trace:
        _cache["last_results"] = res
    return out



# revision 16
# speedup vs baseline: 1.0329x; 1.0329x over previous
# Causal self-attention (B=4, T=2048, C=1024, 16 heads) on 8 NeuronCores.
# Sharding: core = (batch b = core//2) x (head-group hg = core%2, 8 heads each).
# Each core computes its 8 heads' attention for its batch plus the row-slice of
# the output projection; the host sums the two partial projections per batch.
#
# Self-contained: hardcodes shapes; builds + compiles the Bass program once.

import contextlib

import numpy as np
import ml_dtypes

import concourse.bass as bass
import concourse.tile as tile
from concourse import bacc, mybir
from concourse.bass import AP
from concourse.bass_utils import run_bass_kernel_spmd

F32 = mybir.dt.float32
BF16 = mybir.dt.bfloat16
EXP = mybir.ActivationFunctionType.Exp

B, T, C = 4, 2048, 1024
NH, HS = 16, 64
NHPC = 8          # heads per core
D = NHPC * HS     # 512: per-core qkv width
NCORES = 8
TT = T // 128     # 16 token tiles
TC = T // 512     # 4 token chunks
CT = C // 128     # 8 contraction tiles
DT = D // 128     # 4 d-tiles of qT/kT (= head pairs)
PW = 256          # head-pair stride in v tile: [1|0*63|v_e(64)] [1|0*63|v_o(64)]

_cache = {}


def _build():
    nc = bacc.Bacc("TRN2", target_bir_lowering=False, debug=False,
                   num_devices=NCORES)

    xT = nc.dram_tensor("xT", [C, T], BF16, kind="ExternalInput")
    wq = nc.dram_tensor("wq", [C, D], BF16, kind="ExternalInput")
    wk = nc.dram_tensor("wk", [C, D], BF16, kind="ExternalInput")
    wv = nc.dram_tensor("wv", [C, D], BF16, kind="ExternalInput")
    wp = nc.dram_tensor("wp", [D, C], BF16, kind="ExternalInput")
    bq = nc.dram_tensor("bq", [128, DT], F32, kind="ExternalInput")
    bk = nc.dram_tensor("bk", [128, DT], F32, kind="ExternalInput")
    bvbc = nc.dram_tensor("bvbc", [128, D], F32, kind="ExternalInput")
    ypT = nc.dram_tensor("ypT", [C, T], BF16, kind="ExternalOutput")

    with tile.TileContext(nc) as tc, contextlib.ExitStack() as ctx:
        cpool = ctx.enter_context(tc.tile_pool(name="consts", bufs=1))
        xpool = ctx.enter_context(tc.tile_pool(name="x", bufs=1))
        wpool = ctx.enter_context(tc.tile_pool(name="w", bufs=1))
        qkpool = ctx.enter_context(tc.tile_pool(name="qk", bufs=1))
        vpool = ctx.enter_context(tc.tile_pool(name="v", bufs=1))
        opool = ctx.enter_context(tc.tile_pool(name="oT", bufs=1))
        ptpool = ctx.enter_context(tc.tile_pool(name="pt", bufs=8))
        ypool = ctx.enter_context(tc.tile_pool(name="y", bufs=4))
        rrpool = ctx.enter_context(tc.tile_pool(name="rr", bufs=2))
        rvpool = ctx.enter_context(tc.tile_pool(name="rv", bufs=2))
        pfpool = ctx.enter_context(tc.tile_pool(name="pf", bufs=1))
        qkv_ps = ctx.enter_context(
            tc.tile_pool(name="qkv_ps", bufs=2, space="PSUM"))
        s_ps = ctx.enter_context(
            tc.tile_pool(name="s_ps", bufs=2, space="PSUM"))
        o_ps = ctx.enter_context(
            tc.tile_pool(name="o_ps", bufs=2, space="PSUM"))

        # ---- input DMAs: wv0 first (gates the warm-up matmuls), then the
        # tiny biases, then by first compute use ----
        wvt = [wpool.tile([128, D], BF16, tag=f"wv{i}", name=f"wv{i}")
               for i in range(CT)]
        nc.sync.dma_start(wvt[0][:], wv.ap()[0:128, :])
        bqt = cpool.tile([128, DT], F32, tag="bq")
        bkt = cpool.tile([128, DT], F32, tag="bk")
        bvt = cpool.tile([128, D], F32, tag="bv")
        nc.sync.dma_start(bqt[:], bq.ap())
        nc.scalar.dma_start(bkt[:], bk.ap())
        nc.sync.dma_start(bvt[:], bvbc.ap())
        for i in range(1, CT):
            (nc.sync if i % 2 == 0 else nc.scalar).dma_start(
                wvt[i][:], wv.ap()[i * 128:(i + 1) * 128, :])
        xt = [xpool.tile([128, T], BF16, tag=f"xT{i}", name=f"xT{i}")
              for i in range(CT)]
        wqt = [wpool.tile([128, D], BF16, tag=f"wq{i}", name=f"wq{i}")
               for i in range(CT)]
        wkt = [wpool.tile([128, D], BF16, tag=f"wk{i}", name=f"wk{i}")
               for i in range(CT)]

        def dma_x_chunk(cch):
            for i in range(CT):
                (nc.sync if i % 2 == 0 else nc.scalar).dma_start(
                    xt[i][:, cch * 512:(cch + 1) * 512],
                    xT.ap()[i * 128:(i + 1) * 128, cch * 512:(cch + 1) * 512])
        dma_x_chunk(0)
        for i in range(CT):
            nc.sync.dma_start(wqt[i][:], wq.ap()[i * 128:(i + 1) * 128, :])
            nc.scalar.dma_start(wkt[i][:], wk.ap()[i * 128:(i + 1) * 128, :])
        for cch in range(1, TC):
            dma_x_chunk(cch)
        wpt = [wpool.tile([128, C], BF16, tag=f"wp{i}", name=f"wp{i}")
               for i in range(DT)]
        for i in range(DT):
            (nc.sync if i % 2 == 0 else nc.scalar).dma_start(
                wpt[i][:], wp.ap()[i * 128:(i + 1) * 128, :])

        # ---- one-time consts ----
        # tri2: two side-by-side [128,128] upper-tri (keep query>=key) masks
        ones_t = cpool.tile([128, 256], F32, tag="ones_t")
        nc.vector.memset(ones_t[:], 1.0)
        trif = cpool.tile([128, 256], F32, tag="trif")
        nc.gpsimd.affine_select(
            trif[:].rearrange("p (b q) -> p b q", b=2),
            ones_t[:].rearrange("p (b q) -> p b q", b=2),
            pattern=[[0, 2], [1, 128]],
            compare_op=mybir.AluOpType.is_ge, fill=0.0,
            base=0, channel_multiplier=-1)
        tri2 = cpool.tile([128, 256], BF16, tag="tri2")
        nc.vector.tensor_copy(tri2[:], trif[:])

        # ---- warm-up matmuls: keep PE busy (and HAM warm) during input DMA;
        # operand is a memset tile so warm-up never waits on any input DMA
        wrm = cpool.tile([128, 512], BF16, tag="wrm")
        nc.vector.memset(wrm[:], 0.125)
        warm_ps = qkv_ps.tile([128, 512], F32, tag="qkv", name="warmps")
        for w in range(16):
            nc.tensor.matmul(
                warm_ps[:], wrm[:, 0:128], wrm[:, 0:512],
                start=True, stop=True, skip_group_check=True)

        # ---- v tiles per head pair: even block [1|0*63|v_e(64)],
        # odd block [v_o(64)|1|0*63] so odd-head AV output lands on psum
        # partitions 0:64 (aligned with oT rows 0:64 -> no shift DMA)
        vt = [vpool.tile([128, DT * PW], BF16, tag=f"v{j}", name=f"v{j}")
              for j in range(TT)]
        for j in range(TT):
            vre = vt[j][:].rearrange("p (h e w) -> p h e w", e=2, w=128)
            nc.vector.memset(vre[:, :, 0, 0:64], 0.0)
            nc.vector.memset(vre[:, :, 0, 0:1], 1.0)
            nc.vector.memset(vre[:, :, 1, 64:128], 0.0)
            nc.vector.memset(vre[:, :, 1, 64:65], 1.0)

        def v_units(j):
            # one emission unit per matmul; state shared via closure cell
            cell = {}

            def mk(ct):
                def u():
                    if ct == 0:
                        cell["ps"] = qkv_ps.tile([128, D], F32, tag="qkv",
                                                 name="qkvps")
                    nc.tensor.matmul(
                        cell["ps"][:], xt[ct][:, j * 128:(j + 1) * 128],
                        wvt[ct][:], start=(ct == 0), stop=(ct == CT - 1))
                    if ct == CT - 1:
                        ps = cell["ps"]
                        vre = vt[j][:].rearrange("p (h e w) -> p h e w",
                                                 e=2, w=128)
                        pse = ps[:].rearrange("p (h e x) -> p h e x",
                                              e=2, x=64)
                        bve = bvt[:].rearrange("p (h e x) -> p h e x",
                                               e=2, x=64)
                        nc.vector.tensor_add(vre[:, :, 0, 64:128],
                                             pse[:, :, 0, :], bve[:, :, 0, :])
                        nc.vector.tensor_add(vre[:, :, 1, 0:64],
                                             pse[:, :, 1, :], bve[:, :, 1, :])
                return u
            return [mk(ct) for ct in range(CT)]

        # ---- qT/kT/oT tiles ----
        qT = [qkpool.tile([128, T], BF16, tag=f"q{d}", name=f"q{d}")
              for d in range(DT)]
        kT = [qkpool.tile([128, T], BF16, tag=f"k{d}", name=f"k{d}")
              for d in range(DT)]
        oT = [opool.tile([128, T], BF16, tag=f"oT{d}", name=f"oT{d}")
              for d in range(DT)]

        def qk_units(hp, idx):
            c, which = idx // 2, idx % 2
            wt_, bt_, out = ((wqt, bqt, qT), (wkt, bkt, kT))[which]
            cell = {}

            def mk(ct):
                def u():
                    if ct == 0:
                        cell["ps"] = qkv_ps.tile([128, 512], F32, tag="qkv",
                                                 name="qkvps")
                    nc.tensor.matmul(
                        cell["ps"][:], wt_[ct][:, hp * 128:(hp + 1) * 128],
                        xt[ct][:, c * 512:(c + 1) * 512],
                        start=(ct == 0), stop=(ct == CT - 1))
                    if ct == CT - 1:
                        nc.vector.tensor_scalar_add(
                            out[hp][:, c * 512:(c + 1) * 512], cell["ps"][:],
                            bt_[:, hp:hp + 1])
                return u
            return [mk(ct) for ct in range(CT)]

        def proj_units(c):
            units = []
            for o in range(CT):
                cell = {}

                def mk(o, cell, hp):
                    def u():
                        if hp == 0:
                            cell["ps"] = qkv_ps.tile([128, 512], F32,
                                                     tag="qkv", name="qkvps")
                        nc.tensor.matmul(
                            cell["ps"][:], wpt[hp][:, o * 128:(o + 1) * 128],
                            oT[hp][:, c * 512:(c + 1) * 512],
                            start=(hp == 0), stop=(hp == DT - 1))
                        if hp == DT - 1:
                            ys = ypool.tile([128, 512], BF16, tag="y",
                                            name="ys")
                            nc.vector.tensor_copy(ys[:], cell["ps"][:])
                            nc.sync.dma_start(
                                ypT.ap()[o * 128:(o + 1) * 128,
                                         c * 512:(c + 1) * 512], ys[:])
                    return u
                units += [mk(o, cell, hp) for hp in range(DT)]
            return units

        # last chunk: hp0-2 proj partials prefilled as fillers inside the
        # final attention (psum rotates through qkv_ps; partial -> sbuf f32),
        # leaving only the 8 hp3 matmuls + DVE adds for the true tail
        partial_t = [pfpool.tile([128, 512], F32, tag=f"pf{o}", name=f"pf{o}")
                     for o in range(CT)]

        def proj_partial_units(c):
            units = []
            for o in range(CT):
                cell = {}

                def mk(o, cell, hp):
                    def u():
                        if hp == 0:
                            cell["ps"] = qkv_ps.tile([128, 512], F32,
                                                     tag="qkv", name="ppps")
                        nc.tensor.matmul(
                            cell["ps"][:], wpt[hp][:, o * 128:(o + 1) * 128],
                            oT[hp][:, c * 512:(c + 1) * 512],
                            start=(hp == 0), stop=(hp == DT - 2))
                        if hp == DT - 2:
                            nc.vector.tensor_copy(partial_t[o][:],
                                                  cell["ps"][:])
                    return u
                units += [mk(o, cell, hp) for hp in range(DT - 1)]
            return units

        pending = []

        def flush_pending():
            # deferred normalize: by now the broadcasts are long done, so
            # these DVE ops never stall the queue (no head-of-line blocking)
            while pending:
                hp_, cs_, op0_, op1_, rv_ = pending.pop(0)
                nc.vector.tensor_mul(oT[hp_][64:128, cs_], op0_[64:128, :],
                                     rv_[64:128, 0:512])
                nc.vector.tensor_mul(oT[hp_][0:64, cs_], op1_[0:64, :],
                                     rv_[0:64, 512:1024])

        def emit_scores(c, hp, jt):
            t = jt - 4 * c              # >= 0 on diagonal-block tiles
            off = 128 * t if t > 0 else 0
            sp = s_ps.tile([128, 1024], F32, tag="s", name="sps")
            for half in range(2):
                nc.tensor.matmul(
                    sp[:, half * 512 + off:(half + 1) * 512],
                    kT[hp][half * 64:(half + 1) * 64,
                           jt * 128:(jt + 1) * 128],
                    qT[hp][half * 64:(half + 1) * 64,
                           c * 512 + off:(c + 1) * 512],
                    start=True, stop=True)
            return sp, t, off

        def emit_exp_mask(sp, t, off):
            pt = ptpool.tile([128, 1024], BF16, tag="pt", name="pt")
            if off == 0:
                nc.scalar.activation(pt[:], sp[:], EXP, scale=0.125)
            else:
                nc.scalar.activation(
                    pt[:].rearrange("p (h q) -> p h q", h=2)[:, :, off:],
                    sp[:].rearrange("p (h q) -> p h q", h=2)[:, :, off:],
                    EXP, scale=0.125)
            if t >= 0:
                # triangle mask on the first 128 valid queries of each head
                nc.vector.tensor_mul(
                    pt[:].rearrange("p (h q) -> p h q",
                                    h=2)[:, :, off:off + 128],
                    pt[:].rearrange("p (h q) -> p h q",
                                    h=2)[:, :, off:off + 128],
                    tri2[:].rearrange("p (b q) -> p b q", b=2))
            return pt

        def emit_av(hp, jt, njt, pt, t, off, op0, op1):
            for half, op in ((0, op0), (1, op1)):
                # stationary 128 cols (FWL): [ones|zeros63|v(64)]
                st = vt[jt][:, hp * PW + 128 * half:
                            hp * PW + 128 * half + 128]
                nc.tensor.matmul(
                    op[:, off:512], st,
                    pt[:, half * 512 + off:(half + 1) * 512],
                    start=(jt == 0), stop=(jt == njt - 1),
                    skip_group_check=(t > 0))

        def attention(c, hp, fillers):
            njt = 4 * c + 4
            cs = slice(c * 512, (c + 1) * 512)
            flush_pending()
            op0 = o_ps.tile([128, 512], F32, tag="o", name="ops")
            op1 = o_ps.tile([128, 512], F32, tag="o", name="ops")
            # spread filler matmul units (next qk/v/proj) across the jt loop
            # so the PE never starves while waiting on the scalar exp
            nf = len(fillers)
            done = 0
            for jt in range(njt):
                sp, t, off = emit_scores(c, hp, jt)
                pt = emit_exp_mask(sp, t, off)
                # fillers go BETWEEN scores and AV: the in-order PE queue then
                # has ready matmuls to run while exp(jt) computes
                want = nf * (jt + 1) // njt
                while done < want:
                    fillers[done]()
                    done += 1
                emit_av(hp, jt, njt, pt, t, off, op0, op1)
            while done < nf:
                fillers[done]()
                done += 1

            # normalize: even rowsum at partition 0 of op0, odd rowsum at
            # partition 64 of op1; y halves land lane-aligned with oT.
            # partition_broadcast only reads partition 0, so shuffle the odd
            # recip row from partition group 16 (=64..67) down to group 0
            # recip/broadcast only operate correctly at partition 0, so the
            # odd rowsum (psum partition 64) goes: DVE copy (lane 64) ->
            # SBUF->SBUF DMA row down to partition 0 -> recip -> broadcast
            rr = rrpool.tile([128, 1536], F32, tag="rr", name="rr")
            nc.vector.reciprocal_approx_fast(rr[0:1, 0:512], op0[0:1, :])
            nc.vector.tensor_copy(rr[64:65, 512:1024], op1[64:65, :])
            nc.gpsimd.dma_start(rr[0:1, 512:1024], rr[64:65, 512:1024])
            nc.vector.reciprocal_approx_fast(rr[0:1, 1024:1536],
                                             rr[0:1, 512:1024])
            rv = rvpool.tile([128, 1024], F32, tag="rv", name="rv")
            nc.gpsimd.partition_broadcast(rv[:, 0:512], rr[0:1, 0:512])
            nc.gpsimd.partition_broadcast(rv[0:64, 512:1024],
                                          rr[0:1, 1024:1536])
            pending.append((hp, cs, op0, op1, rv))

        # ---- main loop: chunk-outer; filler units (next qk, next-chunk v,
        # prev-chunk proj) are spread inside each attention's jt loop so the
        # in-order PE queue always has ready matmuls while exp runs
        for j in range(4):
            for u in v_units(j):
                u()
        for u in qk_units(0, 0) + qk_units(0, 1):
            u()
        proj_rest = []
        for c in range(TC):
            for hp in range(DT):
                # qk/v fillers first (no oT dependency); proj fillers last so
                # they never block the in-order PE queue on the oT chain
                fillers = []
                if hp < DT - 1:
                    fillers += qk_units(hp + 1, 2 * c)
                    fillers += qk_units(hp + 1, 2 * c + 1)
                    if c == TC - 1 and hp == 2:
                        fillers += proj_rest
                        proj_rest = []
                else:
                    if c + 1 < TC:
                        for j in range(4 * (c + 1), 4 * (c + 1) + 4):
                            fillers += v_units(j)
                        fillers += qk_units(0, 2 * (c + 1))
                        fillers += qk_units(0, 2 * (c + 1) + 1)
                        fillers += proj_rest
                        proj_rest = []
                    else:
                        # last attention: prefill last-chunk proj hp0-2
                        fillers += proj_partial_units(c)
                if hp == 1 and c > 0:
                    p = proj_units(c - 1)
                    if c == TC - 1:
                        fillers += p[:16]
                        proj_rest = p[16:]
                    else:
                        fillers += p
                attention(c, hp, fillers)
        flush_pending()
        # keep PE busy (and HAM warm) while the last attention's normalize
        # chain runs; these have no dependencies and drain before the hp3
        # matmuls below become ready
        for w in range(12):
            warm2 = qkv_ps.tile([128, 512], F32, tag="qkv", name="warm2")
            nc.tensor.matmul(
                warm2[:], wrm[:, 0:128], wrm[:, 0:512],
                start=True, stop=True, skip_group_check=True)
        # tail: per o-block one hp3 matmul, then DVE add with the prefilled
        # hp0-2 partial straight to bf16 output; DMAs alternate queues
        cl = TC - 1
        for o in range(CT):
            tps = qkv_ps.tile([128, 512], F32, tag="qkv", name="tps")
            nc.tensor.matmul(
                tps[:], wpt[DT - 1][:, o * 128:(o + 1) * 128],
                oT[DT - 1][:, cl * 512:(cl + 1) * 512], start=True, stop=True)
            ys = ypool.tile([128, 512], BF16, tag="y", name="ys")
            nc.vector.tensor_add(ys[:], tps[:], partial_t[o][:])
            (nc.sync if o % 2 == 0 else nc.scalar).dma_start(
                ypT.ap()[o * 128:(o + 1) * 128,
                         cl * 512:(cl + 1) * 512], ys[:])

    nc.compile()
    return nc


def _shard_inputs(x, Wk, bk, Wq, bq, Wv, bv, Wp, bp):
    bf = ml_dtypes.bfloat16
    in_maps = []
    for core in range(NCORES):
        b, hg = core // 2, core % 2
        sl = slice(hg * D, (hg + 1) * D)
        # oT rows per head-pair block: [odd head (64) | even head (64)]
        wp_perm = Wp[sl, :].reshape(DT, 2, 64, C)[:, ::-1].reshape(D, C)
        in_maps.append({
            "xT": np.ascontiguousarray(x[b].T).astype(bf),
            "wq": np.ascontiguousarray(Wq[:, sl]).astype(bf),
            "wk": np.ascontiguousarray(Wk[:, sl]).astype(bf),
            "wv": np.ascontiguousarray(Wv[:, sl]).astype(bf),
            "wp": np.ascontiguousarray(wp_perm).astype(bf),
            "bq": np.ascontiguousarray(
                bq[sl].reshape(DT, 128).T).astype(np.float32),
            "bk": np.ascontiguousarray(
                bk[sl].reshape(DT, 128).T).astype(np.float32),
            "bvbc": np.ascontiguousarray(
                np.broadcast_to(bv[sl], (128, D))).astype(np.float32),
        })
    return in_maps


def kernel(x, Wk, bk, Wq, bq, Wv, bv, Wp, bp, _trace=False, _trace_kwargs=None):
    x, Wk, bk, Wq, bq, Wv, bv, Wp, bp = [
        np.asarray(a) for a in (x, Wk, bk, Wq, bq, Wv, bv, Wp, bp)]
    if "nc" not in _cache:
        _cache["nc"] = _build()
    nc = _cache["nc"]
    in_maps = _shard_inputs(x, Wk, bk, Wq, bq, Wv, bv, Wp, bp)
    kw = dict(_trace_kwargs or {})
    res = run_bass_kernel_spmd(nc, in_maps, core_ids=list(range(NCORES)),
                               trace=_trace, **kw)
    out = np.empty((B, T, C), np.float32)
    for b in range(B):
        yp = (res.results[2 * b]["ypT"].astype(np.float32)
              + res.results[2 * b + 1]["ypT"].astype(np.float32))
        out[b] = yp.T + bp[None, :]
    if _trace:
        _cache["last_results"] = res
    return out



# revision 18
# speedup vs baseline: 1.0706x; 1.0365x over previous
# Causal self-attention (B=4, T=2048, C=1024, 16 heads) on 8 NeuronCores.
# Sharding: core = (batch b = core//2) x (head-group hg = core%2, 8 heads each).
# Each core computes its 8 heads' attention for its batch plus the row-slice of
# the output projection; the host sums the two partial projections per batch.
#
# Self-contained: hardcodes shapes; builds + compiles the Bass program once.

import contextlib

import numpy as np
import ml_dtypes

import concourse.bass as bass
import concourse.tile as tile
from concourse import bacc, mybir
from concourse.bass import AP
from concourse.bass_utils import run_bass_kernel_spmd

F32 = mybir.dt.float32
BF16 = mybir.dt.bfloat16
EXP = mybir.ActivationFunctionType.Exp

B, T, C = 4, 2048, 1024
NH, HS = 16, 64
NHPC = 8          # heads per core
D = NHPC * HS     # 512: per-core qkv width
NCORES = 8
TT = T // 128     # 16 token tiles
TC = T // 512     # 4 token chunks
CT = C // 128     # 8 contraction tiles
DT = D // 128     # 4 d-tiles of qT/kT (= head pairs)
PW = 256          # head-pair stride in v tile: [1|0*63|v_e(64)] [1|0*63|v_o(64)]

_cache = {}


def _build():
    nc = bacc.Bacc("TRN2", target_bir_lowering=False, debug=False,
                   num_devices=NCORES)

    xT = nc.dram_tensor("xT", [C, T], BF16, kind="ExternalInput")
    wq = nc.dram_tensor("wq", [C, D], BF16, kind="ExternalInput")
    wk = nc.dram_tensor("wk", [C, D], BF16, kind="ExternalInput")
    wv = nc.dram_tensor("wv", [C, D], BF16, kind="ExternalInput")
    wp = nc.dram_tensor("wp", [D, C], BF16, kind="ExternalInput")
    bq = nc.dram_tensor("bq", [128, DT], F32, kind="ExternalInput")
    bk = nc.dram_tensor("bk", [128, DT], F32, kind="ExternalInput")
    bvbc = nc.dram_tensor("bvbc", [128, D], F32, kind="ExternalInput")
    ypT = nc.dram_tensor("ypT", [C, T], BF16, kind="ExternalOutput")

    with tile.TileContext(nc) as tc, contextlib.ExitStack() as ctx:
        cpool = ctx.enter_context(tc.tile_pool(name="consts", bufs=1))
        xpool = ctx.enter_context(tc.tile_pool(name="x", bufs=1))
        wpool = ctx.enter_context(tc.tile_pool(name="w", bufs=1))
        qkpool = ctx.enter_context(tc.tile_pool(name="qk", bufs=1))
        vpool = ctx.enter_context(tc.tile_pool(name="v", bufs=1))
        opool = ctx.enter_context(tc.tile_pool(name="oT", bufs=1))
        ptpool = ctx.enter_context(tc.tile_pool(name="pt", bufs=8))
        ypool = ctx.enter_context(tc.tile_pool(name="y", bufs=4))
        rrpool = ctx.enter_context(tc.tile_pool(name="rr", bufs=2))
        rvpool = ctx.enter_context(tc.tile_pool(name="rv", bufs=2))
        pfpool = ctx.enter_context(tc.tile_pool(name="pf", bufs=1))
        sypool = ctx.enter_context(tc.tile_pool(name="sy", bufs=1))
        qkv_ps = ctx.enter_context(
            tc.tile_pool(name="qkv_ps", bufs=2, space="PSUM"))
        s_ps = ctx.enter_context(
            tc.tile_pool(name="s_ps", bufs=2, space="PSUM"))
        o_ps = ctx.enter_context(
            tc.tile_pool(name="o_ps", bufs=2, space="PSUM"))

        # ---- input DMAs: wv0 first (gates the warm-up matmuls), then the
        # tiny biases, then by first compute use ----
        wvt = [wpool.tile([128, D], BF16, tag=f"wv{i}", name=f"wv{i}")
               for i in range(CT)]
        nc.sync.dma_start(wvt[0][:], wv.ap()[0:128, :])
        bqt = cpool.tile([128, DT], F32, tag="bq")
        bkt = cpool.tile([128, DT], F32, tag="bk")
        bvt = cpool.tile([128, D], F32, tag="bv")
        nc.sync.dma_start(bqt[:], bq.ap())
        nc.scalar.dma_start(bkt[:], bk.ap())
        nc.sync.dma_start(bvt[:], bvbc.ap())
        for i in range(1, CT):
            (nc.sync if i % 2 == 0 else nc.scalar).dma_start(
                wvt[i][:], wv.ap()[i * 128:(i + 1) * 128, :])
        xt = [xpool.tile([128, T], BF16, tag=f"xT{i}", name=f"xT{i}")
              for i in range(CT)]
        wqt = [wpool.tile([128, D], BF16, tag=f"wq{i}", name=f"wq{i}")
               for i in range(CT)]
        wkt = [wpool.tile([128, D], BF16, tag=f"wk{i}", name=f"wk{i}")
               for i in range(CT)]

        def dma_x_chunk(cch):
            for i in range(CT):
                (nc.sync if i % 2 == 0 else nc.scalar).dma_start(
                    xt[i][:, cch * 512:(cch + 1) * 512],
                    xT.ap()[i * 128:(i + 1) * 128, cch * 512:(cch + 1) * 512])
        dma_x_chunk(0)
        for i in range(CT):
            nc.sync.dma_start(wqt[i][:], wq.ap()[i * 128:(i + 1) * 128, :])
            nc.scalar.dma_start(wkt[i][:], wk.ap()[i * 128:(i + 1) * 128, :])
        for cch in range(1, TC):
            dma_x_chunk(cch)
        wpt = [wpool.tile([128, C], BF16, tag=f"wp{i}", name=f"wp{i}")
               for i in range(DT)]
        for i in range(DT):
            (nc.sync if i % 2 == 0 else nc.scalar).dma_start(
                wpt[i][:], wp.ap()[i * 128:(i + 1) * 128, :])

        # ---- one-time consts ----
        # tri2: two side-by-side [128,128] upper-tri (keep query>=key) masks
        ones_t = cpool.tile([128, 256], F32, tag="ones_t")
        nc.vector.memset(ones_t[:], 1.0)
        trif = cpool.tile([128, 256], F32, tag="trif")
        nc.gpsimd.affine_select(
            trif[:].rearrange("p (b q) -> p b q", b=2),
            ones_t[:].rearrange("p (b q) -> p b q", b=2),
            pattern=[[0, 2], [1, 128]],
            compare_op=mybir.AluOpType.is_ge, fill=0.0,
            base=0, channel_multiplier=-1)
        tri2 = cpool.tile([128, 256], BF16, tag="tri2")
        nc.vector.tensor_copy(tri2[:], trif[:])

        # ---- warm-up matmuls: keep PE busy (and HAM warm) during input DMA;
        # operand is a memset tile so warm-up never waits on any input DMA
        wrm = cpool.tile([128, 512], BF16, tag="wrm")
        nc.vector.memset(wrm[:], 0.125)
        warm_ps = qkv_ps.tile([128, 512], F32, tag="qkv", name="warmps")
        for w in range(16):
            nc.tensor.matmul(
                warm_ps[:], wrm[:, 0:128], wrm[:, 0:512],
                start=True, stop=True, skip_group_check=True)

        # ---- v tiles per head pair: even block [1|0*63|v_e(64)],
        # odd block [v_o(64)|1|0*63] so odd-head AV output lands on psum
        # partitions 0:64 (aligned with oT rows 0:64 -> no shift DMA)
        vt = [vpool.tile([128, DT * PW], BF16, tag=f"v{j}", name=f"v{j}")
              for j in range(TT)]
        for j in range(TT):
            vre = vt[j][:].rearrange("p (h e w) -> p h e w", e=2, w=128)
            nc.vector.memset(vre[:, :, 0, 0:64], 0.0)
            nc.vector.memset(vre[:, :, 0, 0:1], 1.0)
            nc.vector.memset(vre[:, :, 1, 64:128], 0.0)
            nc.vector.memset(vre[:, :, 1, 64:65], 1.0)

        def v_units(j):
            # one emission unit per matmul; state shared via closure cell
            cell = {}

            def mk(ct):
                def u():
                    if ct == 0:
                        cell["ps"] = qkv_ps.tile([128, D], F32, tag="qkv",
                                                 name="qkvps")
                    nc.tensor.matmul(
                        cell["ps"][:], xt[ct][:, j * 128:(j + 1) * 128],
                        wvt[ct][:], start=(ct == 0), stop=(ct == CT - 1))
                    if ct == CT - 1:
                        ps = cell["ps"]
                        vre = vt[j][:].rearrange("p (h e w) -> p h e w",
                                                 e=2, w=128)
                        pse = ps[:].rearrange("p (h e x) -> p h e x",
                                              e=2, x=64)
                        bve = bvt[:].rearrange("p (h e x) -> p h e x",
                                               e=2, x=64)
                        nc.vector.tensor_add(vre[:, :, 0, 64:128],
                                             pse[:, :, 0, :], bve[:, :, 0, :])
                        nc.vector.tensor_add(vre[:, :, 1, 0:64],
                                             pse[:, :, 1, :], bve[:, :, 1, :])
                return u
            return [mk(ct) for ct in range(CT)]

        # ---- qT/kT/oT tiles ----
        qT = [qkpool.tile([128, T], BF16, tag=f"q{d}", name=f"q{d}")
              for d in range(DT)]
        kT = [qkpool.tile([128, T], BF16, tag=f"k{d}", name=f"k{d}")
              for d in range(DT)]
        oT = [opool.tile([128, T], BF16, tag=f"oT{d}", name=f"oT{d}")
              for d in range(DT)]

        def qk_units(hp, idx):
            c, which = idx // 2, idx % 2
            wt_, bt_, out = ((wqt, bqt, qT), (wkt, bkt, kT))[which]
            cell = {}

            def mk(ct):
                def u():
                    if ct == 0:
                        cell["ps"] = qkv_ps.tile([128, 512], F32, tag="qkv",
                                                 name="qkvps")
                    nc.tensor.matmul(
                        cell["ps"][:], wt_[ct][:, hp * 128:(hp + 1) * 128],
                        xt[ct][:, c * 512:(c + 1) * 512],
                        start=(ct == 0), stop=(ct == CT - 1))
                    if ct == CT - 1:
                        nc.vector.tensor_scalar_add(
                            out[hp][:, c * 512:(c + 1) * 512], cell["ps"][:],
                            bt_[:, hp:hp + 1])
                return u
            return [mk(ct) for ct in range(CT)]

        def proj_units(c):
            units = []
            for o in range(CT):
                cell = {}

                def mk(o, cell, hp):
                    def u():
                        if hp == 0:
                            cell["ps"] = qkv_ps.tile([128, 512], F32,
                                                     tag="qkv", name="qkvps")
                        nc.tensor.matmul(
                            cell["ps"][:], wpt[hp][:, o * 128:(o + 1) * 128],
                            oT[hp][:, c * 512:(c + 1) * 512],
                            start=(hp == 0), stop=(hp == DT - 1))
                        if hp == DT - 1:
                            ys = ypool.tile([128, 512], BF16, tag="y",
                                            name="ys")
                            nc.vector.tensor_copy(ys[:], cell["ps"][:])
                            nc.sync.dma_start(
                                ypT.ap()[o * 128:(o + 1) * 128,
                                         c * 512:(c + 1) * 512], ys[:])
                    return u
                units += [mk(o, cell, hp) for hp in range(DT)]
            return units

        # last chunk: hp0-2 proj partials prefilled as fillers inside the
        # final attention (psum rotates through qkv_ps; partial -> sbuf f32),
        # leaving only the 8 hp3 matmuls + DVE adds for the true tail
        partial_t = [pfpool.tile([128, 512], F32, tag=f"pf{o}", name=f"pf{o}")
                     for o in range(CT)]

        def proj_partial_units(c):
            units = []
            for o in range(CT):
                cell = {}

                def mk(o, cell, hp):
                    def u():
                        if hp == 0:
                            cell["ps"] = qkv_ps.tile([128, 512], F32,
                                                     tag="qkv", name="ppps")
                        nc.tensor.matmul(
                            cell["ps"][:], wpt[hp][:, o * 128:(o + 1) * 128],
                            oT[hp][:, c * 512:(c + 1) * 512],
                            start=(hp == 0), stop=(hp == DT - 2))
                        if hp == DT - 2:
                            nc.vector.tensor_copy(partial_t[o][:],
                                                  cell["ps"][:])
                    return u
                units += [mk(o, cell, hp) for hp in range(DT - 1)]
            return units

        pending = []

        def flush_pending():
            # deferred normalize: by now the broadcasts are long done, so
            # these DVE ops never stall the queue (no head-of-line blocking)
            while pending:
                hp_, cs_, sy_, rv_ = pending.pop(0)
                nc.vector.tensor_mul(oT[hp_][64:128, cs_], sy_[64:128, 0:512],
                                     rv_[64:128, 0:512])
                nc.vector.tensor_mul(oT[hp_][0:64, cs_], sy_[0:64, 512:1024],
                                     rv_[0:64, 512:1024])

        def emit_scores(c, hp, jt):
            t = jt - 4 * c              # >= 0 on diagonal-block tiles
            off = 128 * t if t > 0 else 0
            sp = s_ps.tile([128, 1024], F32, tag="s", name="sps")
            for half in range(2):
                nc.tensor.matmul(
                    sp[:, half * 512 + off:(half + 1) * 512],
                    kT[hp][half * 64:(half + 1) * 64,
                           jt * 128:(jt + 1) * 128],
                    qT[hp][half * 64:(half + 1) * 64,
                           c * 512 + off:(c + 1) * 512],
                    start=True, stop=True)
            return sp, t, off

        def emit_exp_mask(sp, t, off):
            pt = ptpool.tile([128, 1024], BF16, tag="pt", name="pt")
            if off == 0:
                nc.scalar.activation(pt[:], sp[:], EXP, scale=0.125)
            else:
                nc.scalar.activation(
                    pt[:].rearrange("p (h q) -> p h q", h=2)[:, :, off:],
                    sp[:].rearrange("p (h q) -> p h q", h=2)[:, :, off:],
                    EXP, scale=0.125)
            if t >= 0:
                # triangle mask on the first 128 valid queries of each head
                nc.vector.tensor_mul(
                    pt[:].rearrange("p (h q) -> p h q",
                                    h=2)[:, :, off:off + 128],
                    pt[:].rearrange("p (h q) -> p h q",
                                    h=2)[:, :, off:off + 128],
                    tri2[:].rearrange("p (b q) -> p b q", b=2))
            return pt

        def emit_av(hp, jt, njt, pt, t, off, op0, op1):
            for half, op in ((0, op0), (1, op1)):
                # stationary 128 cols (FWL): [ones|zeros63|v(64)]
                st = vt[jt][:, hp * PW + 128 * half:
                            hp * PW + 128 * half + 128]
                nc.tensor.matmul(
                    op[:, off:512], st,
                    pt[:, half * 512 + off:(half + 1) * 512],
                    start=(jt == 0), stop=(jt == njt - 1),
                    skip_group_check=(t > 0))

        def attention(c, hp, fillers):
            njt = 4 * c + 4
            cs = slice(c * 512, (c + 1) * 512)
            flush_pending()
            op0 = o_ps.tile([128, 512], F32, tag="o", name="ops")
            op1 = o_ps.tile([128, 512], F32, tag="o", name="ops")
            # spread filler matmul units (next qk/v/proj) across the jt loop
            # so the PE never starves while waiting on the scalar exp
            nf = len(fillers)
            done = 0
            for jt in range(njt):
                sp, t, off = emit_scores(c, hp, jt)
                pt = emit_exp_mask(sp, t, off)
                # fillers go BETWEEN scores and AV: the in-order PE queue then
                # has ready matmuls to run while exp(jt) computes
                want = nf * (jt + 1) // njt
                while done < want:
                    fillers[done]()
                    done += 1
                emit_av(hp, jt, njt, pt, t, off, op0, op1)
            while done < nf:
                fillers[done]()
                done += 1

            # normalize: even rowsum at partition 0 of op0, odd rowsum at
            # partition 64 of op1; y halves land lane-aligned with oT.
            # partition_broadcast only reads partition 0, so shuffle the odd
            # recip row from partition group 16 (=64..67) down to group 0
            # recip/broadcast only operate correctly at partition 0, so the
            # odd rowsum (psum partition 64) goes: DVE copy (lane 64) ->
            # SBUF->SBUF DMA row down to partition 0 -> recip -> broadcast
            rr = rrpool.tile([128, 1536], F32, tag="rr", name="rr")
            nc.vector.reciprocal_approx_fast(rr[0:1, 0:512], op0[0:1, :])
            nc.vector.tensor_copy(rr[64:65, 512:1024], op1[64:65, :])
            nc.gpsimd.dma_start(rr[0:1, 512:1024], rr[64:65, 512:1024])
            nc.vector.reciprocal_approx_fast(rr[0:1, 1024:1536],
                                             rr[0:1, 512:1024])
            rv = rvpool.tile([128, 1024], F32, tag="rv", name="rv")
            nc.gpsimd.partition_broadcast(rv[:, 0:512], rr[0:1, 0:512])
            nc.gpsimd.partition_broadcast(rv[0:64, 512:1024],
                                          rr[0:1, 1024:1536])
            sy = sypool.tile([128, 1024], BF16, tag="sy", name="sy")
            nc.vector.tensor_copy(sy[64:128, 0:512], op0[64:128, :])
            nc.vector.tensor_copy(sy[0:64, 512:1024], op1[0:64, :])
            pending.append((hp, cs, sy, rv))

        # ---- main loop: chunk-outer; filler units (next qk, next-chunk v,
        # prev-chunk proj) are spread inside each attention's jt loop so the
        # in-order PE queue always has ready matmuls while exp runs
        for j in range(4):
            for u in v_units(j):
                u()
        for u in qk_units(0, 0) + qk_units(0, 1):
            u()
        proj_rest = []
        for c in range(TC):
            for hp in range(DT):
                # qk/v fillers first (no oT dependency); proj fillers last so
                # they never block the in-order PE queue on the oT chain
                fillers = []
                if hp < DT - 1:
                    fillers += qk_units(hp + 1, 2 * c)
                    fillers += qk_units(hp + 1, 2 * c + 1)
                    if c == TC - 1 and hp == 2:
                        fillers += proj_rest
                        proj_rest = []
                else:
                    if c + 1 < TC:
                        for j in range(4 * (c + 1), 4 * (c + 1) + 4):
                            fillers += v_units(j)
                        fillers += qk_units(0, 2 * (c + 1))
                        fillers += qk_units(0, 2 * (c + 1) + 1)
                        fillers += proj_rest
                        proj_rest = []
                    else:
                        # last attention: prefill last-chunk proj hp0-2
                        fillers += proj_partial_units(c)
                if hp == 1 and c > 0:
                    p = proj_units(c - 1)
                    if c == TC - 1:
                        fillers += p[:16]
                        proj_rest = p[16:]
                    else:
                        fillers += p
                attention(c, hp, fillers)
        flush_pending()
        # keep PE busy (and HAM warm) while the last attention's normalize
        # chain runs; these have no dependencies and drain before the hp3
        # matmuls below become ready
        for w in range(12):
            warm2 = qkv_ps.tile([128, 512], F32, tag="qkv", name="warm2")
            nc.tensor.matmul(
                warm2[:], wrm[:, 0:128], wrm[:, 0:512],
                start=True, stop=True, skip_group_check=True)
        # tail: per o-block one hp3 matmul, then DVE add with the prefilled
        # hp0-2 partial straight to bf16 output; DMAs alternate queues
        cl = TC - 1
        for o in range(CT):
            tps = qkv_ps.tile([128, 512], F32, tag="qkv", name="tps")
            nc.tensor.matmul(
                tps[:], wpt[DT - 1][:, o * 128:(o + 1) * 128],
                oT[DT - 1][:, cl * 512:(cl + 1) * 512], start=True, stop=True)
            ys = ypool.tile([128, 512], BF16, tag="y", name="ys")
            nc.vector.tensor_add(ys[:], tps[:], partial_t[o][:])
            (nc.sync if o % 2 == 0 else nc.scalar).dma_start(
                ypT.ap()[o * 128:(o + 1) * 128,
                         cl * 512:(cl + 1) * 512], ys[:])

    nc.compile()
    return nc


def _shard_inputs(x, Wk, bk, Wq, bq, Wv, bv, Wp, bp):
    bf = ml_dtypes.bfloat16
    in_maps = []
    for core in range(NCORES):
        b, hg = core // 2, core % 2
        sl = slice(hg * D, (hg + 1) * D)
        # oT rows per head-pair block: [odd head (64) | even head (64)]
        wp_perm = Wp[sl, :].reshape(DT, 2, 64, C)[:, ::-1].reshape(D, C)
        in_maps.append({
            "xT": np.ascontiguousarray(x[b].T).astype(bf),
            "wq": np.ascontiguousarray(Wq[:, sl]).astype(bf),
            "wk": np.ascontiguousarray(Wk[:, sl]).astype(bf),
            "wv": np.ascontiguousarray(Wv[:, sl]).astype(bf),
            "wp": np.ascontiguousarray(wp_perm).astype(bf),
            "bq": np.ascontiguousarray(
                bq[sl].reshape(DT, 128).T).astype(np.float32),
            "bk": np.ascontiguousarray(
                bk[sl].reshape(DT, 128).T).astype(np.float32),
            "bvbc": np.ascontiguousarray(
                np.broadcast_to(bv[sl], (128, D))).astype(np.float32),
        })
    return in_maps


def kernel(x, Wk, bk, Wq, bq, Wv, bv, Wp, bp, _trace=False, _trace_kwargs=None):
    x, Wk, bk, Wq, bq, Wv, bv, Wp, bp = [
        np.asarray(a) for a in (x, Wk, bk, Wq, bq, Wv, bv, Wp, bp)]
    if "nc" not in _cache:
        _cache["nc"] = _build()
    nc = _cache["nc"]
    in_maps = _shard_inputs(x, Wk, bk, Wq, bq, Wv, bv, Wp, bp)
    kw = dict(_trace_kwargs or {})
    res = run_bass_kernel_spmd(nc, in_maps, core_ids=list(range(NCORES)),
                               trace=_trace, **kw)
    out = np.empty((B, T, C), np.float32)
    for b in range(B):
        yp = (res.results[2 * b]["ypT"].astype(np.float32)
              + res.results[2 * b + 1]["ypT"].astype(np.float32))
        out[b] = yp.T + bp[None, :]
    if _trace:
        _cache["last_results"] = res
    return out

